# revision 5
# baseline (speedup 1.0000x reference)
"""AG-GEMM on 8 TRN2 NeuronCores.

Reference computes: A_full[8192, 4096] @ weight.T[4096, 4096] -> [8192, 4096],
where A_full is the concat of 8 per-rank shards A_shards[r] of [1024, 4096].

Strategy: pure row-parallel tensor parallelism. Core r computes
C_r = A_shards[r] @ weight.T with the full weight replicated per core, so no
collective is needed. Host pre-transposes both operands so the contraction
axis (K) lands on SBUF partitions:

  a blob per core  [128, 32*1024]: a[p, kt*1024+m] = A_r[m, kt*128+p]
  w blob (shared)  [32, 128, 4096]: w[nt, p, kt*128+j] = weight[nt*128+j, kt*128+p]

Per core the kernel keeps all of A resident in SBUF (16 MB), streams W
column-blocks (2 MB each, once), and accumulates C^T tiles in PSUM:

  out[nt, j, m] = sum_k w[k, nt*128+j] * a[k, m]   (C^T layout [4096, 1024])

Matmuls run at float32r (full-rate fp32 on the PE when the moving dim >= 256).
"""

import numpy as np

WORLD = 8
M_LOCAL = 1024
K = 4096
N = 4096
KT = K // 128   # 32 k-tiles
NT = N // 128   # 32 n-tiles
MB = M_LOCAL // 512  # 2 moving blocks per k-tile

MM_DTYPE = "float32r"  # set to "float32" to fall back to exact fp32 (4x slower)


def _build_nc():
    from contextlib import ExitStack

    from concourse import bacc, mybir, tile

    f32 = mybir.dt.float32
    mm_dt = getattr(mybir.dt, MM_DTYPE)

    nc = bacc.Bacc("TRN2", target_bir_lowering=False, debug=False)

    a_ext = nc.dram_tensor("a", [128, KT * M_LOCAL], mm_dt, kind="ExternalInput")
    w_ext = nc.dram_tensor("w", [NT, 128, KT * 128], mm_dt, kind="ExternalInput")
    out_ext = nc.dram_tensor("out", [NT, 128, M_LOCAL], f32, kind="ExternalOutput")

    with tile.TileContext(nc) as tc, ExitStack() as ctx:
        a_pool = ctx.enter_context(tc.tile_pool(name="a", bufs=1))
        w_pool = ctx.enter_context(tc.tile_pool(name="w", bufs=2))
        o_pool = ctx.enter_context(tc.tile_pool(name="o", bufs=2))
        ps_pool = ctx.enter_context(tc.tile_pool(name="ps", bufs=4, space="PSUM"))

        # A resident in SBUF, one 512KB DMA per k-tile so early matmuls
        # don't wait for the whole 16MB.
        a_tiles = []
        for kt in range(KT):
            at = a_pool.tile([128, M_LOCAL], mm_dt, name=f"a{kt}", tag=f"a{kt}")
            nc.sync.dma_start(at[:], a_ext[:, kt * M_LOCAL : (kt + 1) * M_LOCAL])
            a_tiles.append(at)

        for nt in range(NT):
            w_sb = w_pool.tile([128, KT * 128], mm_dt)
            nc.sync.dma_start(w_sb[:], w_ext[nt])

            psums = [ps_pool.tile([128, 512], f32, name=f"ps{mb}", tag=f"ps{mb}") for mb in range(MB)]
            for kt in range(KT):
                lhsT = w_sb[:, kt * 128 : (kt + 1) * 128]
                for mb in range(MB):
                    nc.tensor.matmul(
                        psums[mb][:],
                        lhsT,
                        a_tiles[kt][:, mb * 512 : (mb + 1) * 512],
                        start=(kt == 0),
                        stop=(kt == KT - 1),
                    )

            o_sb = o_pool.tile([128, M_LOCAL], f32)
            for mb in range(MB):
                nc.vector.tensor_copy(o_sb[:, mb * 512 : (mb + 1) * 512], psums[mb][:])
            nc.sync.dma_start(out_ext[nt], o_sb[:])

    nc.compile()
    return nc


def _round_tf32(x):
    """Round-to-nearest-even at 10-bit mantissa (TF32 grid) so fp32r HW
    rounding is a no-op on our values."""
    u = np.ascontiguousarray(x, dtype=np.float32).view(np.uint32)
    r = (u + np.uint32(0xFFF) + ((u >> np.uint32(13)) & np.uint32(1))) & np.uint32(0xFFFFE000)
    return r.view(np.float32)


def _prep_inputs(A_shards, weight):
    A_shards = _round_tf32(A_shards) if MM_DTYPE == "float32r" else np.ascontiguousarray(A_shards, dtype=np.float32)
    weight = _round_tf32(weight) if MM_DTYPE == "float32r" else np.ascontiguousarray(weight, dtype=np.float32)

    # w blob: [nt, p, kt*128+j] = weight[nt*128+j, kt*128+p]
    w_blob = np.ascontiguousarray(
        weight.reshape(NT, 128, KT, 128).transpose(0, 3, 2, 1).reshape(NT, 128, KT * 128)
    )

    in_maps = []
    for r in range(WORLD):
        # a blob: [p, kt*1024+m] = A_r[m, kt*128+p]
        a_blob = np.ascontiguousarray(
            A_shards[r].T.reshape(KT, 128, M_LOCAL).transpose(1, 0, 2).reshape(128, KT * M_LOCAL)
        )
        in_maps.append({"a": a_blob, "w": w_blob})
    return in_maps


def _gather_output(results):
    # per-core out [NT, 128, M_LOCAL] is C_r^T tiles: out[nt, j, m] = C_r[m, nt*128+j]
    parts = []
    for r in range(WORLD):
        o = results[r]["out"]
        parts.append(o.transpose(2, 0, 1).reshape(M_LOCAL, N))
    return np.ascontiguousarray(np.concatenate(parts, axis=0))


_NC = None


def _get_nc():
    global _NC
    if _NC is None:
        _NC = _build_nc()
    return _NC


def kernel(A_shards, weight, transed_weight=0, **_ignored):
    from concourse import bass_utils

    nc = _get_nc()
    in_maps = _prep_inputs(A_shards, weight)
    res = bass_utils.run_bass_kernel_spmd(nc, in_maps, core_ids=list(range(WORLD)))
    return _gather_output(res.results)


if __name__ == "__main__":
    rng = np.random.default_rng(0)
    A = rng.standard_normal((WORLD, M_LOCAL, K), dtype=np.float32)
    W = (rng.standard_normal((N, K), dtype=np.float32) * 0.02).astype(np.float32)
    out = kernel(A, W, 0)
    ref = A.reshape(WORLD * M_LOCAL, K) @ W.T
    err = np.abs(out - ref).max() / max(np.abs(ref).max(), 1e-12)
    print("abs-rel err vs local numpy:", err)


# revision 6
# speedup vs baseline: 1.3409x; 1.3409x over previous
"""AG-GEMM on 8 TRN2 NeuronCores.

Reference computes: A_full[8192, 4096] @ weight.T[4096, 4096] -> [8192, 4096],
where A_full is the concat of 8 per-rank shards A_shards[r] of [1024, 4096].

Strategy: pure row-parallel tensor parallelism. Core r computes
C_r = A_shards[r] @ weight.T with the full weight replicated per core, so no
collective is needed. Host pre-transposes both operands so the contraction
axis (K) lands on SBUF partitions:

  a blob per core  [128, 32*1024]: a[p, kt*1024+m] = A_r[m, kt*128+p]
  w blob (shared)  [32, 128, 4096]: w[nt, p, kt*128+j] = weight[nt*128+j, kt*128+p]

Per core the kernel keeps all of A resident in SBUF (16 MB), streams W
column-blocks (2 MB each, once), and accumulates C^T tiles in PSUM:

  out[nt, j, m] = sum_k w[k, nt*128+j] * a[k, m]   (C^T layout [4096, 1024])

Matmuls run at float32r (full-rate fp32 on the PE when the moving dim >= 256).
"""

import numpy as np

WORLD = 8
M_LOCAL = 1024
K = 4096
N = 4096
KT = K // 128   # 32 k-tiles
NT = N // 128   # 32 n-tiles
MB = M_LOCAL // 512  # 2 moving blocks per k-tile

MM_DTYPE = "float16"  # 10-bit mantissa like tf32, full-rate PE, half DMA


def _build_nc():
    from contextlib import ExitStack

    from concourse import bacc, mybir, tile

    f32 = mybir.dt.float32
    mm_dt = getattr(mybir.dt, MM_DTYPE)

    nc = bacc.Bacc("TRN2", target_bir_lowering=False, debug=False)

    a_ext = nc.dram_tensor("a", [128, KT * M_LOCAL], mm_dt, kind="ExternalInput")
    w_ext = nc.dram_tensor("w", [NT, 128, KT * 128], mm_dt, kind="ExternalInput")
    out_ext = nc.dram_tensor("out", [NT, 128, M_LOCAL], f32, kind="ExternalOutput")

    with tile.TileContext(nc) as tc, ExitStack() as ctx:
        a_pool = ctx.enter_context(tc.tile_pool(name="a", bufs=1))
        w_pool = ctx.enter_context(tc.tile_pool(name="w", bufs=2))
        o_pool = ctx.enter_context(tc.tile_pool(name="o", bufs=2))
        ps_pool = ctx.enter_context(tc.tile_pool(name="ps", bufs=4, space="PSUM"))

        WCH = 4  # DMA chunks per W column so early k-tiles start promptly
        wc = KT * 128 // WCH

        def load_w(nt):
            w_sb = w_pool.tile([128, KT * 128], mm_dt, name=f"w{nt}", tag="w")
            for c in range(WCH):
                nc.scalar.dma_start(
                    w_sb[:, c * wc : (c + 1) * wc], w_ext[nt, :, c * wc : (c + 1) * wc]
                )
            return w_sb

        # First W column before the A stream so the PE can start early;
        # W rides the ACT HWDGE ring, A the SP ring, so they don't serialize.
        w_next = load_w(0)

        # A resident in SBUF, one DMA per k-tile so early matmuls don't
        # wait for the whole array.
        a_tiles = []
        for kt in range(KT):
            at = a_pool.tile([128, M_LOCAL], mm_dt, name=f"a{kt}", tag=f"a{kt}")
            nc.sync.dma_start(at[:], a_ext[:, kt * M_LOCAL : (kt + 1) * M_LOCAL])
            a_tiles.append(at)

        for nt in range(NT):
            w_sb = w_next
            if nt + 1 < NT:
                w_next = load_w(nt + 1)

            psums = [ps_pool.tile([128, 512], f32, name=f"ps{mb}", tag=f"ps{mb}") for mb in range(MB)]
            for kt in range(KT):
                lhsT = w_sb[:, kt * 128 : (kt + 1) * 128]
                for mb in range(MB):
                    nc.tensor.matmul(
                        psums[mb][:],
                        lhsT,
                        a_tiles[kt][:, mb * 512 : (mb + 1) * 512],
                        start=(kt == 0),
                        stop=(kt == KT - 1),
                    )

            o_sb = o_pool.tile([128, M_LOCAL], f32)
            for mb in range(MB):
                nc.vector.tensor_copy(o_sb[:, mb * 512 : (mb + 1) * 512], psums[mb][:])
            nc.sync.dma_start(out_ext[nt], o_sb[:])

    nc.compile()
    return nc


def _round_tf32(x):
    """Round-to-nearest-even at 10-bit mantissa (TF32 grid) so fp32r HW
    rounding is a no-op on our values."""
    u = np.ascontiguousarray(x, dtype=np.float32).view(np.uint32)
    r = (u + np.uint32(0xFFF) + ((u >> np.uint32(13)) & np.uint32(1))) & np.uint32(0xFFFFE000)
    return r.view(np.float32)


def _prep_inputs(A_shards, weight):
    if MM_DTYPE == "float32r":
        A_shards = _round_tf32(A_shards)
        weight = _round_tf32(weight)
        np_dt = np.float32
    elif MM_DTYPE == "float16":
        np_dt = np.float16
    else:
        np_dt = np.float32
    A_shards = np.ascontiguousarray(A_shards, dtype=np_dt)
    weight = np.ascontiguousarray(weight, dtype=np_dt)

    # w blob: [nt, p, kt*128+j] = weight[nt*128+j, kt*128+p]
    w_blob = np.ascontiguousarray(
        weight.reshape(NT, 128, KT, 128).transpose(0, 3, 2, 1).reshape(NT, 128, KT * 128)
    )

    in_maps = []
    for r in range(WORLD):
        # a blob: [p, kt*1024+m] = A_r[m, kt*128+p]
        a_blob = np.ascontiguousarray(
            A_shards[r].T.reshape(KT, 128, M_LOCAL).transpose(1, 0, 2).reshape(128, KT * M_LOCAL)
        )
        in_maps.append({"a": a_blob, "w": w_blob})
    return in_maps


def _gather_output(results):
    # per-core out [NT, 128, M_LOCAL] is C_r^T tiles: out[nt, j, m] = C_r[m, nt*128+j]
    parts = []
    for r in range(WORLD):
        o = results[r]["out"]
        parts.append(o.transpose(2, 0, 1).reshape(M_LOCAL, N))
    return np.ascontiguousarray(np.concatenate(parts, axis=0))


_NC = None


def _get_nc():
    global _NC
    if _NC is None:
        _NC = _build_nc()
    return _NC


def kernel(A_shards, weight, transed_weight=0, **_ignored):
    from concourse import bass_utils

    nc = _get_nc()
    in_maps = _prep_inputs(A_shards, weight)
    res = bass_utils.run_bass_kernel_spmd(nc, in_maps, core_ids=list(range(WORLD)))
    return _gather_output(res.results)


if __name__ == "__main__":
    rng = np.random.default_rng(0)
    A = rng.standard_normal((WORLD, M_LOCAL, K), dtype=np.float32)
    W = (rng.standard_normal((N, K), dtype=np.float32) * 0.02).astype(np.float32)
    out = kernel(A, W, 0)
    ref = A.reshape(WORLD * M_LOCAL, K) @ W.T
    err = np.abs(out - ref).max() / max(np.abs(ref).max(), 1e-12)
    print("abs-rel err vs local numpy:", err)
